# revision 40
# baseline (speedup 1.0000x reference)
"""GCNConvSC (residual + GCNConv) Trainium2 Bass kernel, 8-core SPMD. v2

Math (matches the PyG-style reference):
    deg[v]  = indeg_with_selfloop(v)          (count of v in dst, +1)
    u       = deg^{-1/2}
    y       = u[:,None] * x                   (pre-scaled node features)
    z[v]    = sum_{e: dst_e = v} u[v] * y[src_e]
    out[v]  = x[v] + b + (z[v] + u[v]^2 * x[v]) @ W

Device dataflow per core (dst nodes range-partitioned, 12544 slots each):
  - messages y[src] are DMA-gathered as fp8e4m3 rows with a 128-byte payload
    and 256-byte row pitch (y8 staged as [NPAD, 256] fp8, data in cols 0:128).
    The 128B descriptor is charged ~11.4ns (vs 22.8ns for bf16 256B rows),
    halving gather DMA time. The gather instruction is constructed directly
    (the bass wrapper asserts elem_size_bytes%256==0, a pitch constraint that
    the 256B stride already satisfies).
  - scatter into per-window PSUM tiles by one-hot matmuls: oh[e, slot] =
    (iota==slot_e)*u[dst_e] built on DVE in bf16 (2x mode), matmul
    lhsT=fp8 msgs rhs=bf16 one-hot -> f32 psum.
  - per supergroup of SG_W windows: psum + ysT (u^2 x, self loop) -> bf16 tmp,
    out = W^T @ tmp + xsT via a second matmul, stored bf16 and upcast on host.

Edges are sorted by (supergroup, chunk, window) and padded per (chunk,
window) run to a multiple of 128 (pad: src row 0 with slot -1 -> zero
one-hot column). One dma_gather per (supergroup, chunk) segment amortizes
the per-instruction SWDGE overhead. The tile schedule is shared across all
8 cores (SPMD): tiles per bucket = max over cores.
"""

import os
import sys

sys.path.insert(0, "/opt/trn_rl_repo")

import numpy as np

N_NODES = 100000
F = 128
N_CORES = 8
S = 12544            # dst slots per core (98 windows of 128)
WN = 98              # windows per core
SG_W = int(os.environ.get("GCN_SG_W", "8"))  # windows per supergroup
# gather-source chunks (int16 idx => chunk size <= 32768); sized so the
# per-(chunk, window) bucket means land just under tile multiples: big
# chunks ~603 edges/window (5 tiles of 128), runt ~232 (2 tiles)
CHUNK_STARTS = (0, 32768, 65536, 98304)
CHUNK_SIZES = (32768, 32768, 32768, 1792)
N_CHUNKS = 4
NPAD = 100096        # padded node rows for y

# one-hot build engine schedule: v=DVE, p=Pool(gpsimd), a=Act(scalar)
OH_SCHED = os.environ.get("GCN_OH_SCHED", "v" * 14 + "p" * 3 + "a" * 3)
MSGS_BUFS = int(os.environ.get("GCN_MSGS_BUFS", "9"))
PSUM_BUFS = int(os.environ.get("GCN_PSUM_BUFS", "4"))
OHV_BUFS = int(os.environ.get("GCN_OHV_BUFS", "12"))
OHP_BUFS = int(os.environ.get("GCN_OHP_BUFS", "4"))
OHA_BUFS = int(os.environ.get("GCN_OHA_BUFS", "5"))


def _host_plan(edge_index):
    """Sort/bucket edges per core; emit the shared SPMD schedule plus
    per-core gather-index and slot arrays."""
    src = np.asarray(edge_index[0], dtype=np.int64)
    dst = np.asarray(edge_index[1], dtype=np.int64)

    deg_e = np.bincount(dst, minlength=N_NODES)
    u = (1.0 / np.sqrt(deg_e.astype(np.float64) + 1.0)).astype(np.float32)

    # Deal dsts snake-wise by descending degree across cores: every core's
    # position-p dst has ~the same degree, so per-(chunk, window) counts are
    # nearly equal across cores and the shared max-based schedule pads little.
    order = np.argsort(-deg_e, kind="stable")
    i = np.arange(N_NODES)
    blk, lane = i // N_CORES, i % N_CORES
    core_i = np.where(blk % 2 == 0, lane, N_CORES - 1 - lane)
    # perm[c, p] = global dst at (core c, slot position p); -1 = empty slot
    perm = np.full((N_CORES, S), -1, dtype=np.int64)
    perm[core_i, blk] = order
    core_of_node = np.empty(N_NODES, dtype=np.int64)
    pos_of_node = np.empty(N_NODES, dtype=np.int64)
    core_of_node[order] = core_i
    pos_of_node[order] = blk

    core_of = core_of_node[dst]
    pos_e_all = pos_of_node[dst]
    u_e_all = u[dst]
    chunk_of = np.searchsorted(np.asarray(CHUNK_STARTS), src, side="right") - 1

    # per-core, per-(window, chunk) edge lists
    per_core = []
    counts = np.zeros((N_CORES, N_CHUNKS, WN), dtype=np.int64)
    for c in range(N_CORES):
        m = core_of == c
        es, pos_e, ue = src[m], pos_e_all[m], u_e_all[m]
        ch = chunk_of[m]
        w = pos_e // 128
        slot = pos_e % 128
        # sort edges by (supergroup, chunk, window)
        sg = w // SG_W
        so = np.lexsort((w, ch, sg))
        es, slot, ch, w, ue = es[so], slot[so], ch[so], w[so], ue[so]
        np.add.at(counts[c], (ch, w), 1)
        per_core.append((es, slot, ch, w, ue))

    # shared schedule: tiles per (chunk, window) = max over cores
    n_tiles = (counts.max(axis=0) + 127) // 128  # [N_CHUNKS, WN]
    # every window needs >=1 tile overall so its PSUM region gets written
    empty_w = n_tiles.sum(axis=0) == 0
    n_tiles[0, empty_w] = 1

    # schedule: for sg, for chunk, list of (window, n_tiles); plus first/last
    # accumulation touch per window
    n_sg = (WN + SG_W - 1) // SG_W
    sched = []
    T = 0
    for g in range(n_sg):
        ws = list(range(g * SG_W, min((g + 1) * SG_W, WN)))
        first_touch = {}
        last_touch = {}
        segs = []
        for ch in range(N_CHUNKS):
            tl = []
            for w in ws:
                nt = int(n_tiles[ch, w])
                if nt == 0:
                    continue
                tl.append((w, nt))
                if w not in first_touch:
                    first_touch[w] = (ch, w)
                last_touch[w] = (ch, w)
            segs.append(tl)
        sched.append((g, ws, segs, first_touch, last_touch))
        T += int(n_tiles[:, ws].sum())

    # per-core padded edge streams in schedule order
    idx16 = np.zeros((N_CORES, T * 128), dtype=np.int16)
    slots = np.full((N_CORES, T * 128), -1.0, dtype=np.float32)
    uvals = np.zeros((N_CORES, T * 128), dtype=np.float32)
    for c in range(N_CORES):
        es, eslot, ch, w, ue = per_core[c]
        keys = list(zip(w // SG_W, ch, w))
        run_start = {}
        for i, k in enumerate(keys):
            if k not in run_start:
                run_start[k] = i
        run_len = counts[c]
        out_pos = 0
        for g, ws, segs, _, _ in sched:
            for chp in range(N_CHUNKS):
                for wseg, nt in segs[chp]:
                    cnt = int(run_len[chp, wseg])
                    if cnt > 0:
                        i0 = run_start[(g, chp, wseg)]
                        sl = slice(i0, i0 + cnt)
                        local = (es[sl] - CHUNK_STARTS[chp]).astype(np.int16)
                        idx16[c, out_pos : out_pos + cnt] = local
                        slots[c, out_pos : out_pos + cnt] = eslot[sl].astype(
                            np.float32
                        )
                        uvals[c, out_pos : out_pos + cnt] = ue[sl].astype(np.float32)
                    out_pos += nt * 128
        assert out_pos == T * 128

    return u, n_tiles, sched, T, idx16, slots, uvals, perm


def _fp8_gather(nc, out_ap, in_ap, idxs_ap, num_idxs, elem_size, elem_step,
                queue_num=0):
    """dma_gather with a sub-256B payload (fp8 128B rows on a 256B pitch).

    Same instruction the bass wrapper emits, minus its elem_size_bytes%256
    assert (which is a row-pitch constraint; the pitch here is 256B)."""
    import concourse.mybir as mybir

    g = nc.gpsimd
    stride_bytes = elem_step * mybir.dt.size(in_ap.dtype)
    assert stride_bytes % 256 == 0
    _in = g.lower_ap_dma(in_ap, for_custom_bir_dma=True)
    inst = mybir.InstDMAGatherAnt(
        name=g.bass.get_next_instruction_name(),
        ins=[*_in, g.lower_ap(idxs_ap), g.lower_val_access(g.to_reg(num_idxs))],
        outs=[g.lower_ap(out_ap)],
        transpose=False,
        num_idxs=num_idxs,
        elem_size=elem_size,
        stride_bytes_256=stride_bytes // 256,
        gen_mode=0,
        single_packet=False,
        queue_num=queue_num,
        sbuf_tokens_per_rank=0,
        sbuf_free_dim_per_rank=0,
        sbuf_free_dim_pad_per_rank=0,
        sbuf_byte_offset=0,
    )
    return g.add_instruction(inst)


def _build_program(T, sched):
    import concourse.bacc as bacc
    import concourse.mybir as mybir
    from concourse import tile

    fp8 = mybir.dt.float8e4
    bf16 = mybir.dt.bfloat16
    f32 = mybir.dt.float32

    nc = bacc.Bacc(
        "TRN2",
        target_bir_lowering=False,
        debug=False,
        enable_asserts=True,
        num_devices=N_CORES,
    )

    y_d = nc.dram_tensor("y8", [NPAD, 256], fp8, kind="ExternalInput").ap()
    idx_d = nc.dram_tensor("idx16", [128, T * 8], mybir.dt.int16, kind="ExternalInput").ap()
    slots_d = nc.dram_tensor("slots", [128, T], f32, kind="ExternalInput").ap()
    uvals_d = nc.dram_tensor("uvals", [128, T], f32, kind="ExternalInput").ap()
    iota_d = nc.dram_tensor("iota", [128, 128], bf16, kind="ExternalInput").ap()
    iota_neg_d = nc.dram_tensor("iota_neg", [128, 128], bf16, kind="ExternalInput").ap()
    ident_d = nc.dram_tensor("ident", [128, 128], bf16, kind="ExternalInput").ap()
    ysW_d = nc.dram_tensor("ysW", [128, S], bf16, kind="ExternalInput").ap()
    xsT_d = nc.dram_tensor("xsT", [128, S], bf16, kind="ExternalInput").ap()
    w_d = nc.dram_tensor("W", [F, F], bf16, kind="ExternalInput").ap()
    out_d = nc.dram_tensor("outT", [128, S], bf16, kind="ExternalOutput").ap()

    with tile.TileContext(nc) as tc:
        with (
            tc.tile_pool(name="const", bufs=1) as const_p,
            tc.tile_pool(name="msgs", bufs=MSGS_BUFS) as msgs_p,
            tc.tile_pool(name="oh", bufs=OHV_BUFS) as oh_p,
            tc.tile_pool(name="ohp", bufs=OHP_BUFS) as ohp_p,
            tc.tile_pool(name="oha", bufs=OHA_BUFS) as oha_p,
            tc.tile_pool(name="psum", bufs=PSUM_BUFS, space="PSUM") as psum_p,
            tc.tile_pool(name="fin", bufs=2) as fin_p,
            tc.tile_pool(name="fpsum", bufs=2, space="PSUM") as fpsum_p,
        ):
            idx_sb = const_p.tile([128, T * 8], mybir.dt.int16)
            slots_sb = const_p.tile([128, T], f32)
            uvals_sb = const_p.tile([128, T], f32)
            iota_sb = const_p.tile([128, 128], bf16)
            iota_neg_sb = const_p.tile([128, 128], bf16)
            ident_sb = const_p.tile([128, 128], bf16)
            w_sb = const_p.tile([F, F], bf16)
            ysW_sb = const_p.tile([128, S], bf16)
            xsT_sb = const_p.tile([128, S], bf16)

            nc.sync.dma_start(idx_sb[:], idx_d[:])
            nc.sync.dma_start(slots_sb[:], slots_d[:])
            nc.sync.dma_start(uvals_sb[:], uvals_d[:])
            nc.sync.dma_start(iota_sb[:], iota_d[:])
            nc.sync.dma_start(iota_neg_sb[:], iota_neg_d[:])
            nc.sync.dma_start(ident_sb[:], ident_d[:])
            nc.sync.dma_start(w_sb[:], w_d[:])
            nc.sync.dma_start(ysW_sb[:], ysW_d[:])
            nc.sync.dma_start(xsT_sb[:], xsT_d[:])

            uvals_neg_sb = const_p.tile([128, T], f32)
            nc.vector.tensor_scalar_mul(uvals_neg_sb[:], uvals_sb[:], -1.0)

            oh_rr = [0]

            g_tile = 0  # global tile cursor
            for g, ws, segs, first_touch, last_touch in sched:
                nwin = len(ws)
                w0 = ws[0]
                # gather all chunk segments of this supergroup first
                seg_msgs = {}
                seg_base = {}       # global tile index of segment start
                seg_off = {}        # (ch, w) -> tile offset within segment
                for ch in range(N_CHUNKS):
                    seg_tiles = sum(nt for (_, nt) in segs[ch])
                    if seg_tiles == 0:
                        continue
                    n_idx = seg_tiles * 128
                    msgs = msgs_p.tile([128, seg_tiles * 128], fp8, tag="msgs")
                    m3 = msgs[:].rearrange("p (b f) -> p b f", f=F)
                    c0 = CHUNK_STARTS[ch]
                    _fp8_gather(
                        nc, m3,
                        y_d[c0 : c0 + CHUNK_SIZES[ch], 0:F],
                        idx_sb[:, g_tile * 8 : g_tile * 8 + n_idx // 16],
                        n_idx, F, 256,
                    )
                    seg_msgs[ch] = msgs
                    seg_base[ch] = g_tile
                    tt = 0
                    for wseg, nt in segs[ch]:
                        seg_off[(ch, wseg)] = tt
                        tt += nt
                    g_tile += seg_tiles

                # window-major matmuls: each window's accumulation group is
                # contiguous (interleaved groups in PSUM are unsupported)
                tmp = fin_p.tile([128, nwin * 128], bf16, tag="tmp")
                for w in ws:
                    q = w - w0
                    pw = psum_p.tile([128, 128], f32, tag="psum")
                    # self-loop term: psum_w = ysW_w^T via identity matmul
                    nc.tensor.matmul(
                        pw[:],
                        lhsT=ysW_sb[:, w * 128 : (w + 1) * 128],
                        rhs=ident_sb[:],
                        start=True,
                        stop=(last_touch.get(w) is None),
                    )
                    for ch in range(N_CHUNKS):
                        if (ch, w) not in seg_off:
                            continue
                        nt = dict(segs[ch])[w]
                        msgs = seg_msgs[ch]
                        toff = seg_off[(ch, w)]
                        for k in range(nt):
                            gt = seg_base[ch] + toff + k
                            # oh[e, j] = (iota_j == slot_e) * u[dst_e], built
                            # round-robin on DVE / Pool / Act (DVE's ~148ns
                            # per-instruction issue rate is the bottleneck)
                            eng = OH_SCHED[oh_rr[0] % len(OH_SCHED)]
                            oh_rr[0] += 1
                            if eng == "v" or eng == "p":
                                oh = (oh_p if eng == "v" else ohp_p).tile(
                                    [128, 128], bf16)
                                e = nc.vector if eng == "v" else nc.gpsimd
                                e.tensor_scalar(
                                    oh[:],
                                    iota_sb[:],
                                    slots_sb[:, gt : gt + 1],
                                    uvals_sb[:, gt : gt + 1],
                                    mybir.AluOpType.is_equal,
                                    mybir.AluOpType.mult,
                                )
                            else:
                                # Act: u * relu(1 - (slot - j)^2), exact for
                                # integer iota/slots
                                a1 = oha_p.tile([128, 128], bf16, tag="a1")
                                oh = oha_p.tile([128, 128], bf16, tag="oh")
                                nc.scalar.activation(
                                    a1[:], iota_neg_sb[:],
                                    mybir.ActivationFunctionType.Square,
                                    bias=slots_sb[:, gt : gt + 1],
                                )
                                nc.scalar.activation(
                                    oh[:], a1[:],
                                    mybir.ActivationFunctionType.Relu,
                                    bias=uvals_sb[:, gt : gt + 1],
                                    scale=uvals_neg_sb[:, gt : gt + 1],
                                )
                            nc.tensor.matmul(
                                pw[:],
                                lhsT=msgs[:, (toff + k) * 128 : (toff + k + 1) * 128],
                                rhs=oh[:],
                                start=False,
                                stop=(last_touch[w] == (ch, w) and k == nt - 1),
                            )
                    # drain window on the Act engine: tmp = psum (bf16)
                    nc.scalar.copy(
                        out=tmp[:, q * 128 : (q + 1) * 128],
                        in_=pw[:],
                    )
                for h0 in range(0, nwin * 128, 512):
                    n = min(512, nwin * 128 - h0)
                    pf = fpsum_p.tile([128, 512], f32, tag="fps")
                    nc.tensor.matmul(pf[:, :n], lhsT=w_sb[:], rhs=tmp[:, h0 : h0 + n],
                                     start=True, stop=True)
                    ot = fin_p.tile([128, 512], bf16, tag="ot")
                    nc.vector.tensor_tensor(
                        out=ot[:, :n], in0=pf[:, :n],
                        in1=xsT_sb[:, w0 * 128 + h0 : w0 * 128 + h0 + n],
                        op=mybir.AluOpType.add,
                    )
                    nc.sync.dma_start(out_d[:, w0 * 128 + h0 : w0 * 128 + h0 + n],
                                      ot[:, :n])
            assert g_tile == T

    nc.compile()
    return nc


_PROGRAM_CACHE = {}


def _get_program(T, sched_key, sched):
    key = (T, sched_key)
    if key not in _PROGRAM_CACHE:
        _PROGRAM_CACHE[key] = _build_program(T, sched)
    return _PROGRAM_CACHE[key]


def _prepare(x, edge_index, W, b):
    x = np.asarray(x, dtype=np.float32)
    edge_index = np.asarray(edge_index)
    W = np.asarray(W, dtype=np.float32)
    b = np.asarray(b, dtype=np.float32)

    u, n_tiles, sched, T, idx16, slots, uvals, perm = _host_plan(edge_index)

    import ml_dtypes
    y8 = np.zeros((NPAD, 256), dtype=ml_dtypes.float8_e4m3)
    y8[:N_NODES, :F] = (u[:, None] * x).astype(ml_dtypes.float8_e4m3)

    iota = np.tile(np.arange(128, dtype=np.float32), (128, 1)).astype(
        ml_dtypes.bfloat16
    )
    iota_neg = np.tile(-np.arange(128, dtype=np.float32), (128, 1)).astype(
        ml_dtypes.bfloat16
    )
    ident = np.eye(128, dtype=ml_dtypes.bfloat16)

    # staged per-core rows follow the dst permutation; -1 slots stay zero
    u_ext = np.concatenate([u, [0.0]]).astype(np.float32)
    x_ext = np.concatenate([x, np.zeros((1, F), np.float32)], axis=0)
    # self-loop term, already scaled by u[dst]: u^2 * x
    ys_ext = u_ext[:, None] ** 2 * x_ext

    # position p lives at acc/out column p (window p//128, slot p%128)
    colmap = np.arange(S)

    in_maps = []
    for c in range(N_CORES):
        rows = np.empty(S, dtype=np.int64)  # dst ids by acc column
        rows[colmap] = perm[c]
        # idx stream position i -> [i % 16, i // 16]; 16-row block
        # replicated 8x along partitions (one copy per Q7 core group)
        idx_c = np.tile(idx16[c].reshape(-1, 16).T, (8, 1)).copy()  # [128, T*8]
        slots_c = slots[c].reshape(T, 128).T.copy()  # [128, T]
        ys_rows = ys_ext[rows]  # [S, F]
        # slot-major: ysW[p, w*128+j] = ys_rows[w*128+p, j]
        ysW = (
            ys_rows.reshape(WN, 128, F).transpose(1, 0, 2).reshape(128, WN * F)
        ).astype(ml_dtypes.bfloat16)
        xsT = (x_ext[rows] + b[None, :]).T.astype(ml_dtypes.bfloat16)
        in_maps.append(
            {
                "y8": y8,
                "idx16": idx_c,
                "slots": slots_c.astype(np.float32),
                "uvals": uvals[c].reshape(T, 128).T.copy().astype(np.float32),
                "iota": iota,
                "iota_neg": iota_neg,
                "ident": ident,
                "ysW": np.ascontiguousarray(ysW),
                "xsT": np.ascontiguousarray(xsT),
                "W": W.astype(ml_dtypes.bfloat16),
            }
        )

    sched_key = tuple(
        (g, tuple(ws), tuple(tuple(tuple(t) for t in seg) for seg in segs))
        for g, ws, segs, _, _ in sched
    )
    nc = _get_program(T, sched_key, sched)
    global _LAST_PERM
    colrows = np.empty((N_CORES, S), dtype=np.int64)
    for c in range(N_CORES):
        colrows[c][colmap] = perm[c]
    _LAST_PERM = colrows
    return nc, in_maps


_LAST_PERM = None


def _unshard(results, perm=None):
    if perm is None:
        perm = _LAST_PERM
    out = np.empty((N_NODES, F), dtype=np.float32)
    for c in range(N_CORES):
        rows = perm[c]
        valid = rows >= 0
        out[rows[valid]] = results[c]["outT"].T.astype(np.float32)[valid]
    return out


def kernel(x, edge_index, W, b):
    from concourse.bass_utils import run_bass_kernel_spmd

    nc, in_maps = _prepare(x, edge_index, W, b)
    res = run_bass_kernel_spmd(nc, in_maps, list(range(N_CORES)))
    return _unshard(res.results)


if __name__ == "__main__":
    rng = np.random.default_rng(0)
    x = rng.standard_normal((N_NODES, F), dtype=np.float32)
    ei = rng.integers(0, N_NODES, size=(2, 1600000)).astype(np.int64)
    W = rng.standard_normal((F, F), dtype=np.float32) / np.sqrt(F)
    b = np.zeros(F, dtype=np.float32)
    out = kernel(x=x, edge_index=ei, W=W, b=b)
    print(out.shape, out.dtype)


# revision 41
# speedup vs baseline: 1.1813x; 1.1813x over previous
"""GCNConvSC (residual + GCNConv) Trainium2 Bass kernel, 8-core SPMD. v2

Math (matches the PyG-style reference):
    deg[v]  = indeg_with_selfloop(v)          (count of v in dst, +1)
    u       = deg^{-1/2}
    y       = u[:,None] * x                   (pre-scaled node features)
    z[v]    = sum_{e: dst_e = v} u[v] * y[src_e]
    out[v]  = x[v] + b + (z[v] + u[v]^2 * x[v]) @ W

Device dataflow per core (dst nodes range-partitioned, 12544 slots each):
  - messages y[src] are DMA-gathered as fp8e4m3 rows with a 128-byte payload
    and 256-byte row pitch (y8 staged as [NPAD, 256] fp8, data in cols 0:128).
    The 128B descriptor is charged ~11.4ns (vs 22.8ns for bf16 256B rows),
    halving gather DMA time. The gather instruction is constructed directly
    (the bass wrapper asserts elem_size_bytes%256==0, a pitch constraint that
    the 256B stride already satisfies).
  - scatter into per-window PSUM tiles by one-hot matmuls: oh[e, slot] =
    (iota==slot_e)*u[dst_e] built on DVE in bf16 (2x mode), matmul
    lhsT=fp8 msgs rhs=bf16 one-hot -> f32 psum.
  - per supergroup of SG_W windows: psum + ysT (u^2 x, self loop) -> bf16 tmp,
    out = W^T @ tmp + xsT via a second matmul, stored bf16 and upcast on host.

Edges are sorted by (supergroup, chunk, window) and padded per (chunk,
window) run to a multiple of 128 (pad: src row 0 with slot -1 -> zero
one-hot column). One dma_gather per (supergroup, chunk) segment amortizes
the per-instruction SWDGE overhead. The tile schedule is shared across all
8 cores (SPMD): tiles per bucket = max over cores.
"""

import os
import sys

sys.path.insert(0, "/opt/trn_rl_repo")

import numpy as np

N_NODES = 100000
F = 128
N_CORES = 8
S = 12544            # dst slots per core (98 windows of 128)
WN = 98              # windows per core
SG_W = int(os.environ.get("GCN_SG_W", "8"))  # windows per supergroup
# gather-source chunks (int16 idx => chunk size <= 32768); sized so the
# per-(chunk, window) bucket means land just under tile multiples: big
# chunks ~603 edges/window (5 tiles of 128), runt ~232 (2 tiles)
CHUNK_STARTS = (0, 32768, 65536, 98304)
CHUNK_SIZES = (32768, 32768, 32768, 1792)
N_CHUNKS = 4
NPAD = 100096        # padded node rows for y

# one-hot build engine schedule: v=DVE, p=Pool(gpsimd), a=Act(scalar)
OH_SCHED = os.environ.get("GCN_OH_SCHED", "vvvpavvvvpavvvvpavvv")
MSGS_BUFS = int(os.environ.get("GCN_MSGS_BUFS", "9"))
PSUM_BUFS = int(os.environ.get("GCN_PSUM_BUFS", "4"))
OHV_BUFS = int(os.environ.get("GCN_OHV_BUFS", "12"))
OHP_BUFS = int(os.environ.get("GCN_OHP_BUFS", "4"))
OHA_BUFS = int(os.environ.get("GCN_OHA_BUFS", "5"))


def _host_plan(edge_index):
    """Sort/bucket edges per core; emit the shared SPMD schedule plus
    per-core gather-index and slot arrays."""
    src = np.asarray(edge_index[0], dtype=np.int64)
    dst = np.asarray(edge_index[1], dtype=np.int64)

    deg_e = np.bincount(dst, minlength=N_NODES)
    u = (1.0 / np.sqrt(deg_e.astype(np.float64) + 1.0)).astype(np.float32)

    # Deal dsts snake-wise by descending degree across cores: every core's
    # position-p dst has ~the same degree, so per-(chunk, window) counts are
    # nearly equal across cores and the shared max-based schedule pads little.
    order = np.argsort(-deg_e, kind="stable")
    i = np.arange(N_NODES)
    blk, lane = i // N_CORES, i % N_CORES
    core_i = np.where(blk % 2 == 0, lane, N_CORES - 1 - lane)
    # perm[c, p] = global dst at (core c, slot position p); -1 = empty slot
    perm = np.full((N_CORES, S), -1, dtype=np.int64)
    perm[core_i, blk] = order
    core_of_node = np.empty(N_NODES, dtype=np.int64)
    pos_of_node = np.empty(N_NODES, dtype=np.int64)
    core_of_node[order] = core_i
    pos_of_node[order] = blk

    core_of = core_of_node[dst]
    pos_e_all = pos_of_node[dst]
    u_e_all = u[dst]
    chunk_of = np.searchsorted(np.asarray(CHUNK_STARTS), src, side="right") - 1

    # per-core, per-(window, chunk) edge lists
    per_core = []
    counts = np.zeros((N_CORES, N_CHUNKS, WN), dtype=np.int64)
    for c in range(N_CORES):
        m = core_of == c
        es, pos_e, ue = src[m], pos_e_all[m], u_e_all[m]
        ch = chunk_of[m]
        w = pos_e // 128
        slot = pos_e % 128
        # sort edges by (supergroup, chunk, window)
        sg = w // SG_W
        so = np.lexsort((w, ch, sg))
        es, slot, ch, w, ue = es[so], slot[so], ch[so], w[so], ue[so]
        np.add.at(counts[c], (ch, w), 1)
        per_core.append((es, slot, ch, w, ue))

    # shared schedule: tiles per (chunk, window) = max over cores
    n_tiles = (counts.max(axis=0) + 127) // 128  # [N_CHUNKS, WN]
    # every window needs >=1 tile overall so its PSUM region gets written
    empty_w = n_tiles.sum(axis=0) == 0
    n_tiles[0, empty_w] = 1

    # schedule: for sg, for chunk, list of (window, n_tiles); plus first/last
    # accumulation touch per window
    n_sg = (WN + SG_W - 1) // SG_W
    sched = []
    T = 0
    for g in range(n_sg):
        ws = list(range(g * SG_W, min((g + 1) * SG_W, WN)))
        first_touch = {}
        last_touch = {}
        segs = []
        for ch in range(N_CHUNKS):
            tl = []
            for w in ws:
                nt = int(n_tiles[ch, w])
                if nt == 0:
                    continue
                tl.append((w, nt))
                if w not in first_touch:
                    first_touch[w] = (ch, w)
                last_touch[w] = (ch, w)
            segs.append(tl)
        sched.append((g, ws, segs, first_touch, last_touch))
        T += int(n_tiles[:, ws].sum())

    # per-core padded edge streams in schedule order
    idx16 = np.zeros((N_CORES, T * 128), dtype=np.int16)
    slots = np.full((N_CORES, T * 128), -1.0, dtype=np.float32)
    uvals = np.zeros((N_CORES, T * 128), dtype=np.float32)
    for c in range(N_CORES):
        es, eslot, ch, w, ue = per_core[c]
        keys = list(zip(w // SG_W, ch, w))
        run_start = {}
        for i, k in enumerate(keys):
            if k not in run_start:
                run_start[k] = i
        run_len = counts[c]
        out_pos = 0
        for g, ws, segs, _, _ in sched:
            for chp in range(N_CHUNKS):
                for wseg, nt in segs[chp]:
                    cnt = int(run_len[chp, wseg])
                    if cnt > 0:
                        i0 = run_start[(g, chp, wseg)]
                        sl = slice(i0, i0 + cnt)
                        local = (es[sl] - CHUNK_STARTS[chp]).astype(np.int16)
                        idx16[c, out_pos : out_pos + cnt] = local
                        slots[c, out_pos : out_pos + cnt] = eslot[sl].astype(
                            np.float32
                        )
                        uvals[c, out_pos : out_pos + cnt] = ue[sl].astype(np.float32)
                    out_pos += nt * 128
        assert out_pos == T * 128

    return u, n_tiles, sched, T, idx16, slots, uvals, perm


def _fp8_gather(nc, out_ap, in_ap, idxs_ap, num_idxs, elem_size, elem_step,
                queue_num=0):
    """dma_gather with a sub-256B payload (fp8 128B rows on a 256B pitch).

    Same instruction the bass wrapper emits, minus its elem_size_bytes%256
    assert (which is a row-pitch constraint; the pitch here is 256B)."""
    import concourse.mybir as mybir

    g = nc.gpsimd
    stride_bytes = elem_step * mybir.dt.size(in_ap.dtype)
    assert stride_bytes % 256 == 0
    _in = g.lower_ap_dma(in_ap, for_custom_bir_dma=True)
    inst = mybir.InstDMAGatherAnt(
        name=g.bass.get_next_instruction_name(),
        ins=[*_in, g.lower_ap(idxs_ap), g.lower_val_access(g.to_reg(num_idxs))],
        outs=[g.lower_ap(out_ap)],
        transpose=False,
        num_idxs=num_idxs,
        elem_size=elem_size,
        stride_bytes_256=stride_bytes // 256,
        gen_mode=0,
        single_packet=False,
        queue_num=queue_num,
        sbuf_tokens_per_rank=0,
        sbuf_free_dim_per_rank=0,
        sbuf_free_dim_pad_per_rank=0,
        sbuf_byte_offset=0,
    )
    return g.add_instruction(inst)


def _build_program(T, sched):
    import concourse.bacc as bacc
    import concourse.mybir as mybir
    from concourse import tile

    fp8 = mybir.dt.float8e4
    bf16 = mybir.dt.bfloat16
    f32 = mybir.dt.float32

    nc = bacc.Bacc(
        "TRN2",
        target_bir_lowering=False,
        debug=False,
        enable_asserts=True,
        num_devices=N_CORES,
    )

    y_d = nc.dram_tensor("y8", [NPAD, 256], fp8, kind="ExternalInput").ap()
    idx_d = nc.dram_tensor("idx16", [128, T * 8], mybir.dt.int16, kind="ExternalInput").ap()
    slots_d = nc.dram_tensor("slots", [128, T], f32, kind="ExternalInput").ap()
    uvals_d = nc.dram_tensor("uvals", [128, T], f32, kind="ExternalInput").ap()
    iota_d = nc.dram_tensor("iota", [128, 128], bf16, kind="ExternalInput").ap()
    iota_neg_d = nc.dram_tensor("iota_neg", [128, 128], bf16, kind="ExternalInput").ap()
    ident_d = nc.dram_tensor("ident", [128, 128], bf16, kind="ExternalInput").ap()
    ysW_d = nc.dram_tensor("ysW", [128, S], bf16, kind="ExternalInput").ap()
    xsT_d = nc.dram_tensor("xsT", [128, S], bf16, kind="ExternalInput").ap()
    w_d = nc.dram_tensor("W", [F, F], bf16, kind="ExternalInput").ap()
    out_d = nc.dram_tensor("outT", [128, S], bf16, kind="ExternalOutput").ap()

    with tile.TileContext(nc) as tc:
        with (
            tc.tile_pool(name="const", bufs=1) as const_p,
            tc.tile_pool(name="msgs", bufs=MSGS_BUFS) as msgs_p,
            tc.tile_pool(name="oh", bufs=OHV_BUFS) as oh_p,
            tc.tile_pool(name="ohp", bufs=OHP_BUFS) as ohp_p,
            tc.tile_pool(name="oha", bufs=OHA_BUFS) as oha_p,
            tc.tile_pool(name="psum", bufs=PSUM_BUFS, space="PSUM") as psum_p,
            tc.tile_pool(name="fin", bufs=2) as fin_p,
            tc.tile_pool(name="fpsum", bufs=2, space="PSUM") as fpsum_p,
        ):
            idx_sb = const_p.tile([128, T * 8], mybir.dt.int16)
            slots_sb = const_p.tile([128, T], f32)
            uvals_sb = const_p.tile([128, T], f32)
            iota_sb = const_p.tile([128, 128], bf16)
            iota_neg_sb = const_p.tile([128, 128], bf16)
            ident_sb = const_p.tile([128, 128], bf16)
            w_sb = const_p.tile([F, F], bf16)
            ysW_sb = const_p.tile([128, S], bf16)
            xsT_sb = const_p.tile([128, S], bf16)

            nc.sync.dma_start(idx_sb[:], idx_d[:])
            nc.sync.dma_start(slots_sb[:], slots_d[:])
            nc.sync.dma_start(uvals_sb[:], uvals_d[:])
            nc.sync.dma_start(iota_sb[:], iota_d[:])
            nc.sync.dma_start(iota_neg_sb[:], iota_neg_d[:])
            nc.sync.dma_start(ident_sb[:], ident_d[:])
            nc.sync.dma_start(w_sb[:], w_d[:])
            nc.sync.dma_start(ysW_sb[:], ysW_d[:])
            nc.sync.dma_start(xsT_sb[:], xsT_d[:])

            uvals_neg_sb = const_p.tile([128, T], f32)
            nc.vector.tensor_scalar_mul(uvals_neg_sb[:], uvals_sb[:], -1.0)

            oh_rr = [0]

            g_tile = 0  # global tile cursor
            for g, ws, segs, first_touch, last_touch in sched:
                nwin = len(ws)
                w0 = ws[0]
                # gather all chunk segments of this supergroup first
                seg_msgs = {}
                seg_base = {}       # global tile index of segment start
                seg_off = {}        # (ch, w) -> tile offset within segment
                for ch in range(N_CHUNKS):
                    seg_tiles = sum(nt for (_, nt) in segs[ch])
                    if seg_tiles == 0:
                        continue
                    n_idx = seg_tiles * 128
                    msgs = msgs_p.tile([128, seg_tiles * 128], fp8, tag="msgs")
                    m3 = msgs[:].rearrange("p (b f) -> p b f", f=F)
                    c0 = CHUNK_STARTS[ch]
                    _fp8_gather(
                        nc, m3,
                        y_d[c0 : c0 + CHUNK_SIZES[ch], 0:F],
                        idx_sb[:, g_tile * 8 : g_tile * 8 + n_idx // 16],
                        n_idx, F, 256,
                    )
                    seg_msgs[ch] = msgs
                    seg_base[ch] = g_tile
                    tt = 0
                    for wseg, nt in segs[ch]:
                        seg_off[(ch, wseg)] = tt
                        tt += nt
                    g_tile += seg_tiles

                # window-major matmuls: each window's accumulation group is
                # contiguous (interleaved groups in PSUM are unsupported)
                tmp = fin_p.tile([128, nwin * 128], bf16, tag="tmp")
                for w in ws:
                    q = w - w0
                    pw = psum_p.tile([128, 128], f32, tag="psum")
                    # self-loop term: psum_w = ysW_w^T via identity matmul
                    nc.tensor.matmul(
                        pw[:],
                        lhsT=ysW_sb[:, w * 128 : (w + 1) * 128],
                        rhs=ident_sb[:],
                        start=True,
                        stop=(last_touch.get(w) is None),
                    )
                    for ch in range(N_CHUNKS):
                        if (ch, w) not in seg_off:
                            continue
                        nt = dict(segs[ch])[w]
                        msgs = seg_msgs[ch]
                        toff = seg_off[(ch, w)]
                        for k in range(nt):
                            gt = seg_base[ch] + toff + k
                            # oh[e, j] = (iota_j == slot_e) * u[dst_e], built
                            # round-robin on DVE / Pool / Act (DVE's ~148ns
                            # per-instruction issue rate is the bottleneck)
                            eng = OH_SCHED[oh_rr[0] % len(OH_SCHED)]
                            oh_rr[0] += 1
                            if eng == "v" or eng == "p":
                                oh = (oh_p if eng == "v" else ohp_p).tile(
                                    [128, 128], bf16)
                                e = nc.vector if eng == "v" else nc.gpsimd
                                e.tensor_scalar(
                                    oh[:],
                                    iota_sb[:],
                                    slots_sb[:, gt : gt + 1],
                                    uvals_sb[:, gt : gt + 1],
                                    mybir.AluOpType.is_equal,
                                    mybir.AluOpType.mult,
                                )
                            else:
                                # Act: u * relu(1 - (slot - j)^2), exact for
                                # integer iota/slots
                                a1 = oha_p.tile([128, 128], bf16, tag="a1")
                                oh = oha_p.tile([128, 128], bf16, tag="oh")
                                nc.scalar.activation(
                                    a1[:], iota_neg_sb[:],
                                    mybir.ActivationFunctionType.Square,
                                    bias=slots_sb[:, gt : gt + 1],
                                )
                                nc.scalar.activation(
                                    oh[:], a1[:],
                                    mybir.ActivationFunctionType.Relu,
                                    bias=uvals_sb[:, gt : gt + 1],
                                    scale=uvals_neg_sb[:, gt : gt + 1],
                                )
                            nc.tensor.matmul(
                                pw[:],
                                lhsT=msgs[:, (toff + k) * 128 : (toff + k + 1) * 128],
                                rhs=oh[:],
                                start=False,
                                stop=(last_touch[w] == (ch, w) and k == nt - 1),
                            )
                    # drain window on the Act engine: tmp = psum (bf16)
                    nc.scalar.copy(
                        out=tmp[:, q * 128 : (q + 1) * 128],
                        in_=pw[:],
                    )
                for h0 in range(0, nwin * 128, 512):
                    n = min(512, nwin * 128 - h0)
                    pf = fpsum_p.tile([128, 512], f32, tag="fps")
                    nc.tensor.matmul(pf[:, :n], lhsT=w_sb[:], rhs=tmp[:, h0 : h0 + n],
                                     start=True, stop=True)
                    ot = fin_p.tile([128, 512], bf16, tag="ot")
                    nc.vector.tensor_tensor(
                        out=ot[:, :n], in0=pf[:, :n],
                        in1=xsT_sb[:, w0 * 128 + h0 : w0 * 128 + h0 + n],
                        op=mybir.AluOpType.add,
                    )
                    nc.sync.dma_start(out_d[:, w0 * 128 + h0 : w0 * 128 + h0 + n],
                                      ot[:, :n])
            assert g_tile == T

    nc.compile()
    return nc


_PROGRAM_CACHE = {}


def _get_program(T, sched_key, sched):
    key = (T, sched_key)
    if key not in _PROGRAM_CACHE:
        _PROGRAM_CACHE[key] = _build_program(T, sched)
    return _PROGRAM_CACHE[key]


def _prepare(x, edge_index, W, b):
    x = np.asarray(x, dtype=np.float32)
    edge_index = np.asarray(edge_index)
    W = np.asarray(W, dtype=np.float32)
    b = np.asarray(b, dtype=np.float32)

    u, n_tiles, sched, T, idx16, slots, uvals, perm = _host_plan(edge_index)

    import ml_dtypes
    y8 = np.zeros((NPAD, 256), dtype=ml_dtypes.float8_e4m3)
    y8[:N_NODES, :F] = (u[:, None] * x).astype(ml_dtypes.float8_e4m3)

    iota = np.tile(np.arange(128, dtype=np.float32), (128, 1)).astype(
        ml_dtypes.bfloat16
    )
    iota_neg = np.tile(-np.arange(128, dtype=np.float32), (128, 1)).astype(
        ml_dtypes.bfloat16
    )
    ident = np.eye(128, dtype=ml_dtypes.bfloat16)

    # staged per-core rows follow the dst permutation; -1 slots stay zero
    u_ext = np.concatenate([u, [0.0]]).astype(np.float32)
    x_ext = np.concatenate([x, np.zeros((1, F), np.float32)], axis=0)
    # self-loop term, already scaled by u[dst]: u^2 * x
    ys_ext = u_ext[:, None] ** 2 * x_ext

    # position p lives at acc/out column p (window p//128, slot p%128)
    colmap = np.arange(S)

    in_maps = []
    for c in range(N_CORES):
        rows = np.empty(S, dtype=np.int64)  # dst ids by acc column
        rows[colmap] = perm[c]
        # idx stream position i -> [i % 16, i // 16]; 16-row block
        # replicated 8x along partitions (one copy per Q7 core group)
        idx_c = np.tile(idx16[c].reshape(-1, 16).T, (8, 1)).copy()  # [128, T*8]
        slots_c = slots[c].reshape(T, 128).T.copy()  # [128, T]
        ys_rows = ys_ext[rows]  # [S, F]
        # slot-major: ysW[p, w*128+j] = ys_rows[w*128+p, j]
        ysW = (
            ys_rows.reshape(WN, 128, F).transpose(1, 0, 2).reshape(128, WN * F)
        ).astype(ml_dtypes.bfloat16)
        xsT = (x_ext[rows] + b[None, :]).T.astype(ml_dtypes.bfloat16)
        in_maps.append(
            {
                "y8": y8,
                "idx16": idx_c,
                "slots": slots_c.astype(np.float32),
                "uvals": uvals[c].reshape(T, 128).T.copy().astype(np.float32),
                "iota": iota,
                "iota_neg": iota_neg,
                "ident": ident,
                "ysW": np.ascontiguousarray(ysW),
                "xsT": np.ascontiguousarray(xsT),
                "W": W.astype(ml_dtypes.bfloat16),
            }
        )

    sched_key = tuple(
        (g, tuple(ws), tuple(tuple(tuple(t) for t in seg) for seg in segs))
        for g, ws, segs, _, _ in sched
    )
    nc = _get_program(T, sched_key, sched)
    global _LAST_PERM
    colrows = np.empty((N_CORES, S), dtype=np.int64)
    for c in range(N_CORES):
        colrows[c][colmap] = perm[c]
    _LAST_PERM = colrows
    return nc, in_maps


_LAST_PERM = None


def _unshard(results, perm=None):
    if perm is None:
        perm = _LAST_PERM
    out = np.empty((N_NODES, F), dtype=np.float32)
    for c in range(N_CORES):
        rows = perm[c]
        valid = rows >= 0
        out[rows[valid]] = results[c]["outT"].T.astype(np.float32)[valid]
    return out


def kernel(x, edge_index, W, b):
    from concourse.bass_utils import run_bass_kernel_spmd

    nc, in_maps = _prepare(x, edge_index, W, b)
    res = run_bass_kernel_spmd(nc, in_maps, list(range(N_CORES)))
    return _unshard(res.results)


if __name__ == "__main__":
    rng = np.random.default_rng(0)
    x = rng.standard_normal((N_NODES, F), dtype=np.float32)
    ei = rng.integers(0, N_NODES, size=(2, 1600000)).astype(np.int64)
    W = rng.standard_normal((F, F), dtype=np.float32) / np.sqrt(F)
    b = np.zeros(F, dtype=np.float32)
    out = kernel(x=x, edge_index=ei, W=W, b=b)
    print(out.shape, out.dtype)
